# revision 11
# baseline (speedup 1.0000x reference)
"""Trainium2 Bass kernel for multi-head attention (B=8, N=1024, C=768, H=12).

Strategy: data-parallel over batch — core b computes batch element b entirely
locally (no collectives). Host prepares transposed bf16 inputs; device does
QKV^T, scores S[k,q] (softmax without max-subtraction — scores ~N(0,1), exp is
safe), exp on ACT directly from PSUM, attn@V with a ones-column for the
softmax denominators, reciprocal + PE-broadcast division, output projection.

v3 schedule:
- p-major host layouts so every DMA row is 2-3KB contiguous on both sides;
  per-ct xT DMAs + per-mpair wqk DMAs interleaved across the SP and ACT
  hardware queues in dependency order
- PE warm-fill matmuls (zeroed tile, idle pbig pool) cover the DMA-paced
  head window and keep the p-state ramp hot
- attn@V bursts emitted before scores in each slot; qk tiles in late slots
- pair-packed reciprocal broadcast (12 instead of 24 sel matmuls)
- output projection split: ct0-3 partials during the last pair, ct4-5
  closers pipelined with the final divide (per-qb halves); bf16 output
"""

import numpy as np
import ml_dtypes

B, N, C = 8, 1024, 768
H, HD = 12, 64
SCALE = HD ** -0.5
CT = C // 128   # 6 c-tiles
NT = N // 128   # 8 seq tiles
QB = 2          # q blocks of 512
PAIRS = H // 2  # 6 head pairs


def build_nc():
    import concourse.bass as bass
    import concourse.mybir as mybir
    import concourse.tile as tile
    from concourse import bacc
    from contextlib import ExitStack

    BF = mybir.dt.bfloat16
    F32 = mybir.dt.float32
    EXP = mybir.ActivationFunctionType.Exp

    nc = bacc.Bacc()
    xTt = nc.declare_dram_parameter("xTt", [128, CT * N], BF, isOutput=False)
    wqk = nc.declare_dram_parameter("wqk", [PAIRS, 128, 1536], BF, isOutput=False)
    wvT = nc.declare_dram_parameter("wvT", [128, CT * C], BF, isOutput=False)
    wpT = nc.declare_dram_parameter("wpT", [128, CT * C], BF, isOutput=False)
    sel2 = nc.declare_dram_parameter("sel2", [12, PAIRS * 128], BF, isOutput=False)
    out = nc.declare_dram_parameter("out", [N, C], BF, isOutput=True)

    with tile.TileContext(nc, pool_alloc_mode="queue") as tc, ExitStack() as ctx:
        sb = ctx.enter_context(tc.tile_pool(name="sb", bufs=1))
        ptp = ctx.enter_context(tc.tile_pool(name="pt", bufs=22))
        attp = ctx.enter_context(tc.tile_pool(name="attp", bufs=3))
        yp = ctx.enter_context(tc.tile_pool(name="y", bufs=4))
        tmpp = ctx.enter_context(tc.tile_pool(name="tmp", bufs=4))
        pbig = ctx.enter_context(tc.tile_pool(name="pbig", bufs=3, space="PSUM"))
        psmall = ctx.enter_context(tc.tile_pool(name="psmall", bufs=2, space="PSUM"))

        # ---- persistent SBUF tiles
        xT_sb = sb.tile([128, CT * N], BF, tag="xT")            # ct at cols ct*1024
        wqk_sb = sb.tile([128, PAIRS * 1536], BF, tag="wqk")    # m-pair mp at cols mp*1536; ct at +ct*256
        wv_sb = sb.tile([128, CT * C], BF, tag="wv")            # ct at cols ct*768
        wp_sb = sb.tile([128, CT * C], BF, tag="wp")
        qkT_sb = sb.tile([128, 12 * N], BF, tag="qkT")          # m-tile mt at cols mt*1024
        ves_sb = sb.tile([128, NT * 13 * 65], BF, tag="ves")    # kt at cols kt*845; head h at +h*65, ones col at +64; pad head slot 12
        attf_sb = sb.tile([128, CT * N], BF, tag="attf")        # divided attn output, c-major layout
        den_sb = sb.tile([12, N], BF, tag="den")
        denf_sb = sb.tile([12, N], F32, tag="denf")
        recip_sb = sb.tile([12, N], F32, tag="recip")
        rb_sb = sb.tile([12, N], BF, tag="rb")                  # bf16 recip for PE broadcast
        sel2_sb = sb.tile([12, PAIRS * 128], BF, tag="sel2")    # pair-packed one-hot selectors
        yA_sb = sb.tile([128, NT * C], F32, tag="yA")           # proj ct0-3 partials, nt at cols nt*768
        warm_sb = sb.tile([128, 640], BF, tag="warm")

        VS = 13 * 65  # 845 cols per kt block in ves (12 heads x 65 + padding)

        # ---- PE warm-up/warm-fill machinery: matmuls on a zeroed tile keep
        # the PE busy (p-state ramp + fill) while input DMAs stream
        nc.gpsimd.memset(warm_sb[:, :], 0.0)
        warmf_sb = sb.tile([1, 16], F32, tag="warmf")
        nc.gpsimd.memset(warmf_sb[:, :], 0.0)
        nc.scalar.activation(warmf_sb[:, :], warmf_sb[:, :], EXP)  # preload exp table set

        def warm_fill(n):
            for _ in range(n):
                ps = pbig.tile([128, 512], F32, tag="big", name="warm")
                nc.tensor.matmul(ps[:, :], lhsT=warm_sb[:, 0:128], rhs=warm_sb[:, 128:640],
                                 start=True, stop=True, skip_group_check=True)

        warm_fill(10)

        # ---- input DMAs: contiguous p-major rows, two queues, dependency order
        def dma_xt(engine, ct):
            engine.dma_start(out=xT_sb[:, ct * N:(ct + 1) * N], in_=xTt[:, ct * N:(ct + 1) * N])

        def dma_wqk(engine, mp):
            engine.dma_start(out=wqk_sb[:, mp * 1536:(mp + 1) * 1536], in_=wqk[mp])

        dma_xt(nc.sync, 0)
        dma_xt(nc.scalar, 1)
        dma_xt(nc.sync, 2)
        dma_xt(nc.scalar, 3)
        dma_xt(nc.sync, 4)
        dma_xt(nc.scalar, 5)
        dma_wqk(nc.sync, 0)
        dma_wqk(nc.scalar, 1)
        nc.sync.dma_start(out=wv_sb[:, :], in_=wvT[:, :])
        dma_wqk(nc.scalar, 2)
        dma_wqk(nc.sync, 3)
        dma_wqk(nc.scalar, 4)
        dma_wqk(nc.sync, 5)
        nc.scalar.dma_start(out=wp_sb[:, :], in_=wpT[:, :])
        nc.sync.dma_start(out=sel2_sb[:, :], in_=sel2[:, :])

        nc.gpsimd.memset(den_sb[:, :], 1.0)
        for kt in range(NT):
            vv = ves_sb[:, kt * VS:(kt + 1) * VS].rearrange("p (h e) -> p h e", e=65)
            nc.gpsimd.memset(vv[:, 0:12, 64:65], 1.0)

        # ---- helpers
        QK_ORD = [0, 6, 1, 7, 2, 8, 3, 9, 4, 10, 5, 11]

        def qk_mtile_half(mt, qb, warm_between=0):
            pos = QK_ORD.index(mt)
            mp, i = pos // 2, pos % 2
            ps = psmall.tile([128, 512], F32, tag="mm", name=f"qk{mt}_{qb}")
            for ct in range(CT):
                nc.tensor.matmul(
                    ps[:, :],
                    lhsT=wqk_sb[:, mp * 1536 + ct * 256 + i * 128: mp * 1536 + ct * 256 + i * 128 + 128],
                    rhs=xT_sb[:, ct * N + qb * 512: ct * N + qb * 512 + 512],
                    start=(ct == 0), stop=(ct == CT - 1),
                    skip_group_check=(warm_between > 0),
                )
                if warm_between and ct < CT - 1:
                    warm_fill(warm_between)
            nc.vector.tensor_copy(qkT_sb[:, mt * N + qb * 512: mt * N + qb * 512 + 512], ps[:, :])

        def v_ntile(nt):
            """Compute V natural rows [nt*128, +128] and scatter into ves (+ones cols)."""
            for vb in range(2):
                ps = psmall.tile([128, 512], F32, tag="mm", name=f"v{nt}_{vb}")
                for ct in range(CT):
                    nc.tensor.matmul(
                        ps[:, 0:384],
                        lhsT=xT_sb[:, ct * N + nt * 128: ct * N + (nt + 1) * 128],
                        rhs=wv_sb[:, ct * C + vb * 384: ct * C + (vb + 1) * 384],
                        start=(ct == 0), stop=(ct == CT - 1),
                    )
                dst = ves_sb[:, nt * VS:(nt + 1) * VS].rearrange("p (h e) -> p h e", e=65)
                nc.vector.tensor_copy(
                    dst[:, vb * 6:(vb + 1) * 6, 0:64],
                    ps[:, 0:384].rearrange("p (h e) -> p h e", e=64),
                )

        def q_slice(h, qb):
            po = (h % 2) * 64
            return qkT_sb[po:po + 64, (h // 2) * N + qb * 512: (h // 2) * N + qb * 512 + 512]

        def q_full(h):
            po = (h % 2) * 64
            return qkT_sb[po:po + 64, (h // 2) * N: (h // 2) * N + N]

        def k_slice(h, kt):
            po = (h % 2) * 64
            base = (6 + h // 2) * N + kt * 128
            return qkT_sb[po:po + 64, base: base + 128]

        # pipeline state
        y_tiles = {}
        pt_kt = {}         # (pair, kt, j) -> [128, 1024] bf16 exp tile
        att_tiles = {}     # pair -> [128, 2048] bf16 (rows 0-63 numerators, row 64 denominators)

        def scores_and_exp(p, kt):
            h0, h1 = 2 * p, 2 * p + 1
            ps0 = pbig.tile([128, 1024], F32, tag="big")
            ps1 = pbig.tile([128, 1024], F32, tag="big")
            for qb in range(QB):
                nc.tensor.matmul(ps0[:, qb * 512: qb * 512 + 512], lhsT=k_slice(h0, kt),
                                 rhs=q_slice(h0, qb), start=True, stop=True)
                nc.tensor.matmul(ps1[:, qb * 512: qb * 512 + 512], lhsT=k_slice(h1, kt),
                                 rhs=q_slice(h1, qb), start=True, stop=True)
            pt0 = ptp.tile([128, 1024], BF, tag="pt", name=f"pt{p}_{kt}a")
            pt1 = ptp.tile([128, 1024], BF, tag="pt", name=f"pt{p}_{kt}b")
            pt_kt[(p, kt, 0)], pt_kt[(p, kt, 1)] = pt0, pt1
            nc.scalar.activation(pt0[:, :], ps0[:, :], EXP)
            nc.scalar.activation(pt1[:, :], ps1[:, :], EXP)

        po_open = {}

        def attn_burst_half(p, j, qb, second, evac=None):
            """Half of an attn@V accumulation group (4 MMs). Group closes and
            evacuates on the second half; den row DMAs per (j, qb) half."""
            h = 2 * p + j
            if not second:
                po_open[(p, j, qb)] = psmall.tile([65, 512], F32, tag="mm", name=f"po{h}_{qb}")
            po = po_open[(p, j, qb)]
            k0 = 4 if second else 0
            for kt in range(k0, k0 + 4):
                nc.tensor.matmul(
                    po[:, :],
                    lhsT=ves_sb[:, kt * VS + h * 65: kt * VS + h * 65 + 65],
                    rhs=pt_kt[(p, kt, j)][:, qb * 512: qb * 512 + 512],
                    start=(kt == 0), stop=(kt == NT - 1),
                )
            if second:
                po_open.pop((p, j, qb))
                if p not in att_tiles:
                    att_tiles[p] = attp.tile([128, 2 * N], BF, tag="att", name=f"att{p}")
                att_t = att_tiles[p]
                if evac == "act":
                    nc.scalar.copy(
                        att_t[0:65, j * 1024 + qb * 512: j * 1024 + qb * 512 + 512], po[:, :])
                else:
                    nc.vector.tensor_copy(
                        att_t[0:65, j * 1024 + qb * 512: j * 1024 + qb * 512 + 512], po[:, :])
                nc.sync.dma_start(
                    out=den_sb[h:h + 1, qb * 512: qb * 512 + 512],
                    in_=att_t[64:65, j * 1024 + qb * 512: j * 1024 + qb * 512 + 512])
                if qb == 1:
                    for kt in range(NT):
                        pt_kt.pop((p, kt, j))

        def attn_burst(p, j, qb, evac=None):
            attn_burst_half(p, j, qb, False)
            attn_burst_half(p, j, qb, True, evac=evac)

        def recip_half(qb):
            # full-tile ops: partition bases other than 0/32/64/96 are illegal,
            # so recompute all 12 rows (unwritten rows hold memset 1.0)
            s = slice(qb * 512, qb * 512 + 512)
            nc.vector.tensor_copy(denf_sb[:, s], den_sb[:, s])
            nc.vector.reciprocal_approx_fast(recip_sb[:, s], denf_sb[:, s])
            nc.vector.tensor_copy(rb_sb[:, s], recip_sb[:, s])

        def recip_pair(p):
            recip_half(0)
            recip_half(1)

        def divide_qb(p, qb):
            """One broadcast matmul covers both heads of the pair: bc rows 0-63
            hold recip[2p], rows 64-127 hold recip[2p+1]."""
            att_t = att_tiles[p]
            bc = psmall.tile([128, 512], F32, tag="mm", name=f"bc{p}_{qb}")
            nc.tensor.matmul(
                bc[:, :],
                lhsT=sel2_sb[0:12, p * 128:(p + 1) * 128],
                rhs=rb_sb[0:12, qb * 512: qb * 512 + 512],
                start=True, stop=True)
            for j in range(2):
                po = j * 64
                nc.vector.tensor_mul(
                    attf_sb[po:po + 64, p * N + qb * 512: p * N + qb * 512 + 512],
                    att_t[0:64, j * 1024 + qb * 512: j * 1024 + qb * 512 + 512],
                    bc[po:po + 64, :])
            if qb == 1:
                att_tiles.pop(p)

        def proj_a(nt, mb, evac=None, pool=None):
            """ct0-3 partial of the output projection -> yA."""
            if pool is None:
                pool = psmall
            tg = "mm" if pool is psmall else "big"
            ps = pool.tile([128, 512], F32, tag=tg, name=f"ya{nt}_{mb}")
            for ct in range(4):
                nc.tensor.matmul(
                    ps[:, 0:384],
                    lhsT=attf_sb[:, ct * N + nt * 128: ct * N + (nt + 1) * 128],
                    rhs=wp_sb[:, ct * C + mb * 384: ct * C + (mb + 1) * 384],
                    start=(ct == 0), stop=(ct == 3),
                )
            if evac == "act":
                nc.scalar.copy(yA_sb[:, nt * C + mb * 384: nt * C + (mb + 1) * 384], ps[:, 0:384])
            else:
                nc.vector.tensor_copy(yA_sb[:, nt * C + mb * 384: nt * C + (mb + 1) * 384], ps[:, 0:384])

        def proj_b(nt, mb, k):
            """ct4-5 closer + add partial + bf16 output DMA."""
            ps = pbig.tile([128, 512], F32, tag="big", name=f"yb{nt}_{mb}")
            for ci, ct in enumerate((4, 5)):
                nc.tensor.matmul(
                    ps[:, 0:384],
                    lhsT=attf_sb[:, ct * N + nt * 128: ct * N + (nt + 1) * 128],
                    rhs=wp_sb[:, ct * C + mb * 384: ct * C + (mb + 1) * 384],
                    start=(ci == 0), stop=(ci == 1),
                )
            if mb == 0:
                y_tiles[nt] = yp.tile([128, 768], BF, tag="y", name=f"y{nt}")
            y_t = y_tiles[nt]
            if k % 2 == 0 or nt >= 6:
                nc.vector.tensor_add(y_t[:, mb * 384:(mb + 1) * 384],
                                     yA_sb[:, nt * C + mb * 384: nt * C + (mb + 1) * 384], ps[:, 0:384])
            else:
                tmp = tmpp.tile([128, 384], F32, tag="tmp", name=f"t{nt}_{mb}")
                nc.scalar.copy(tmp[:, :], ps[:, 0:384])
                nc.gpsimd.tensor_add(y_t[:, mb * 384:(mb + 1) * 384],
                                     yA_sb[:, nt * C + mb * 384: nt * C + (mb + 1) * 384], tmp[:, :])
            if mb == 1:
                deng = nc.sync if (nt % 2 == 0 or nt == 7) else nc.scalar
                deng.dma_start(out=out[nt * 128:(nt + 1) * 128, :], in_=y_t[:, :])

        # ---- emission schedule
        # head: qk tiles for pair 0 with warm-fill between DMA-paced matmuls
        qk_mtile_half(0, 0, warm_between=2)
        qk_mtile_half(6, 0, warm_between=1)
        qk_mtile_half(0, 1)
        qk_mtile_half(6, 1)
        warm_fill(10)

        # pair 0: scores early (V weights still in flight), V + qk tiles late
        for kt in range(NT):
            scores_and_exp(0, kt)
            if kt >= 4:
                v_ntile(2 * (kt - 4))
                v_ntile(2 * (kt - 4) + 1)
            if kt == 6:
                qk_mtile_half(1, 0)
                qk_mtile_half(1, 1)
            elif kt == 7:
                qk_mtile_half(7, 0)
                qk_mtile_half(7, 1)

        # pairs 1..5: attn@V of p-1 first in each slot, then scores of p, then
        # qk tiles for p+1 in late slots (pairs 1-4) or proj partials (pair 5)
        for p in range(1, PAIRS):
            last = (p == PAIRS - 1)
            for kt in range(NT):
                if kt < 4:
                    attn_burst(p - 1, kt // 2, kt % 2)
                elif kt == 4:
                    recip_pair(p - 1)
                elif kt in (5, 6):
                    divide_qb(p - 1, kt - 5)
                scores_and_exp(p, kt)
                if not last:
                    if kt >= 4:
                        mt, qb = [(p + 1, 0), (p + 1, 1), (p + 7, 0), (p + 7, 1)][kt - 4]
                        qk_mtile_half(mt, qb)
                else:
                    if kt >= 2:
                        proj_a(kt - 2, 0)
                        proj_a(kt - 2, 1)

        # ---- drain: qb0 attn groups first so the divide chain and the proj
        # closers pipeline per qb half
        p = PAIRS - 1
        attn_burst(p, 0, 0)
        attn_burst(p, 1, 0)
        recip_half(0)
        divide_qb(p, 0)
        proj_a(6, 0, pool=pbig)
        proj_a(6, 1, pool=pbig)
        attn_burst(p, 0, 1)
        attn_burst(p, 1, 1)
        proj_a(7, 0, pool=pbig)
        proj_a(7, 1, pool=pbig)
        k = 0
        for nt in range(4):
            for mb in range(2):
                proj_b(nt, mb, k)
                k += 1
        recip_half(1)
        divide_qb(p, 1)
        for nt in range(4, NT):
            for mb in range(2):
                proj_b(nt, mb, k)
                k += 1

    nc.compile()
    return nc


_CACHE = {}


def _prep_inputs(x, w_qkv, w_proj):
    bf = ml_dtypes.bfloat16
    w = np.array(w_qkv, dtype=np.float32, copy=True)
    w[:C] *= SCALE
    wqkT = w[:2 * C].T.astype(bf)                                # [C, 2C]
    ord_ = [0, 6, 1, 7, 2, 8, 3, 9, 4, 10, 5, 11]
    # p-major m-pair blocks: wqk[mp, p, ct*256 + i*128 + c] with m-tiles
    # (ord_[2mp], ord_[2mp+1])
    wqk = np.zeros((PAIRS, 128, 1536), dtype=bf)
    for b in range(PAIRS):
        for ct in range(CT):
            for i in range(2):
                mt = ord_[2 * b + i]
                wqk[b, :, ct * 256 + i * 128: ct * 256 + (i + 1) * 128] = \
                    wqkT[ct * 128:(ct + 1) * 128, mt * 128:(mt + 1) * 128]
    # p-major [128, ct*C] images of wv/wp: row p, block ct = wT[ct*128+p, :]
    wvT_n = w[2 * C:].T.astype(bf)                               # [C, C]
    wpT_n = np.asarray(w_proj).T.astype(np.float32).astype(bf)   # [C, C]
    wvT_t = np.ascontiguousarray(wvT_n.reshape(CT, 128, C).transpose(1, 0, 2).reshape(128, CT * C))
    wpT_t = np.ascontiguousarray(wpT_n.reshape(CT, 128, C).transpose(1, 0, 2).reshape(128, CT * C))
    sel2 = np.zeros((12, PAIRS * 128), dtype=bf)
    for p in range(PAIRS):
        sel2[2 * p, p * 128:p * 128 + 64] = 1.0
        sel2[2 * p + 1, p * 128 + 64:(p + 1) * 128] = 1.0
    maps = []
    for b in range(B):
        xT = np.asarray(x[b]).T.astype(bf)                       # [C, N]
        xTt = np.ascontiguousarray(xT.reshape(CT, 128, N).transpose(1, 0, 2).reshape(128, CT * N))
        maps.append({
            "xTt": xTt, "wqk": wqk, "wvT": wvT_t, "wpT": wpT_t, "sel2": sel2,
        })
    return maps


def kernel(x, w_qkv, w_proj, b_proj):
    from concourse.bass_utils import run_bass_kernel_spmd

    if "nc" not in _CACHE:
        _CACHE["nc"] = build_nc()
    nc = _CACHE["nc"]
    in_maps = _prep_inputs(x, w_qkv, w_proj)
    res = run_bass_kernel_spmd(nc, in_maps, core_ids=list(range(B)))
    y = np.stack([np.asarray(res.results[i]["out"], dtype=np.float32) for i in range(B)])
    y = y + np.asarray(b_proj, dtype=np.float32)[None, None, :]
    return y.astype(np.float32)


if __name__ == "__main__":
    nc = build_nc()
    print("build OK")


# revision 12
# speedup vs baseline: 1.1949x; 1.1949x over previous
"""Trainium2 Bass kernel for multi-head attention (B=8, N=1024, C=768, H=12).

Strategy: data-parallel over batch — core b computes batch element b entirely
locally (no collectives). Host prepares transposed bf16 inputs; device does
QKV^T, scores S[k,q] (softmax without max-subtraction — scores ~N(0,1), exp is
safe), exp on ACT directly from PSUM, attn@V with a ones-column for the
softmax denominators, reciprocal + PE-broadcast division, output projection.

v3 schedule:
- p-major host layouts so every DMA row is 2-3KB contiguous on both sides;
  per-ct xT DMAs + per-mpair wqk DMAs interleaved across the SP and ACT
  hardware queues in dependency order
- PE warm-fill matmuls (zeroed tile, idle pbig pool) cover the DMA-paced
  head window and keep the p-state ramp hot
- attn@V bursts emitted before scores in each slot; qk tiles in late slots
- pair-packed reciprocal broadcast (12 instead of 24 sel matmuls)
- output projection split: ct0-3 partials during the last pair, ct4-5
  closers pipelined with the final divide (per-qb halves); bf16 output
"""

import numpy as np
import ml_dtypes

B, N, C = 8, 1024, 768
H, HD = 12, 64
SCALE = HD ** -0.5
CT = C // 128   # 6 c-tiles
NT = N // 128   # 8 seq tiles
QB = 2          # q blocks of 512
PAIRS = H // 2  # 6 head pairs


def build_nc():
    import concourse.bass as bass
    import concourse.mybir as mybir
    import concourse.tile as tile
    from concourse import bacc
    from contextlib import ExitStack

    BF = mybir.dt.bfloat16
    F32 = mybir.dt.float32
    EXP = mybir.ActivationFunctionType.Exp

    nc = bacc.Bacc()
    xTt = nc.declare_dram_parameter("xTt", [128, CT * N], BF, isOutput=False)
    wqk = nc.declare_dram_parameter("wqk", [PAIRS, 128, 1536], BF, isOutput=False)
    wvT = nc.declare_dram_parameter("wvT", [128, CT * C], BF, isOutput=False)
    wpT = nc.declare_dram_parameter("wpT", [128, CT * C], BF, isOutput=False)
    sel2 = nc.declare_dram_parameter("sel2", [12, PAIRS * 128], BF, isOutput=False)
    out = nc.declare_dram_parameter("out", [N, C], BF, isOutput=True)

    with tile.TileContext(nc, pool_alloc_mode="queue") as tc, ExitStack() as ctx:
        sb = ctx.enter_context(tc.tile_pool(name="sb", bufs=1))
        ptp = ctx.enter_context(tc.tile_pool(name="pt", bufs=22))
        attp = ctx.enter_context(tc.tile_pool(name="attp", bufs=3))
        yp = ctx.enter_context(tc.tile_pool(name="y", bufs=4))
        tmpp = ctx.enter_context(tc.tile_pool(name="tmp", bufs=4))
        pbig = ctx.enter_context(tc.tile_pool(name="pbig", bufs=3, space="PSUM"))
        psmall = ctx.enter_context(tc.tile_pool(name="psmall", bufs=2, space="PSUM"))

        # ---- persistent SBUF tiles
        xT_sb = sb.tile([128, CT * N], BF, tag="xT")            # ct at cols ct*1024
        wqk_sb = sb.tile([128, PAIRS * 1536], BF, tag="wqk")    # m-pair mp at cols mp*1536; ct at +ct*256
        wv_sb = sb.tile([128, CT * C], BF, tag="wv")            # ct at cols ct*768
        wp_sb = sb.tile([128, CT * C], BF, tag="wp")
        qkT_sb = sb.tile([128, 12 * N], BF, tag="qkT")          # m-tile mt at cols mt*1024
        ves_sb = sb.tile([128, NT * 13 * 65], BF, tag="ves")    # kt at cols kt*845; head h at +h*65, ones col at +64; pad head slot 12
        attf_sb = sb.tile([128, CT * N], BF, tag="attf")        # divided attn output, c-major layout
        den_sb = sb.tile([12, N], BF, tag="den")
        denf_sb = sb.tile([12, N], F32, tag="denf")
        recip_sb = sb.tile([12, N], F32, tag="recip")
        rb_sb = sb.tile([12, N], BF, tag="rb")                  # bf16 recip for PE broadcast
        sel2_sb = sb.tile([12, PAIRS * 128], BF, tag="sel2")    # pair-packed one-hot selectors
        yA_sb = sb.tile([128, NT * C], F32, tag="yA")           # proj ct0-3 partials, nt at cols nt*768
        warm_sb = sb.tile([128, 640], BF, tag="warm")

        VS = 13 * 65  # 845 cols per kt block in ves (12 heads x 65 + padding)

        # ---- PE warm-up/warm-fill machinery: matmuls on a zeroed tile keep
        # the PE busy (p-state ramp + fill) while input DMAs stream
        nc.gpsimd.memset(warm_sb[:, :], 0.0)
        warmf_sb = sb.tile([1, 16], F32, tag="warmf")
        nc.gpsimd.memset(warmf_sb[:, :], 0.0)
        nc.scalar.activation(warmf_sb[:, :], warmf_sb[:, :], EXP)  # preload exp table set

        def warm_fill(n):
            for _ in range(n):
                ps = pbig.tile([128, 512], F32, tag="big", name="warm")
                nc.tensor.matmul(ps[:, :], lhsT=warm_sb[:, 0:128], rhs=warm_sb[:, 128:640],
                                 start=True, stop=True, skip_group_check=True)

        warm_fill(10)

        # ---- input DMAs: contiguous p-major rows, two queues, dependency order
        def dma_xt(engine, ct):
            engine.dma_start(out=xT_sb[:, ct * N:(ct + 1) * N], in_=xTt[:, ct * N:(ct + 1) * N])

        def dma_wqk(engine, mp):
            engine.dma_start(out=wqk_sb[:, mp * 1536:(mp + 1) * 1536], in_=wqk[mp])

        dma_xt(nc.sync, 0)
        dma_xt(nc.scalar, 1)
        dma_xt(nc.sync, 2)
        dma_xt(nc.scalar, 3)
        dma_xt(nc.sync, 4)
        dma_xt(nc.scalar, 5)
        dma_wqk(nc.sync, 0)
        dma_wqk(nc.scalar, 1)
        nc.sync.dma_start(out=wv_sb[:, :], in_=wvT[:, :])
        dma_wqk(nc.scalar, 2)
        dma_wqk(nc.sync, 3)
        dma_wqk(nc.scalar, 4)
        dma_wqk(nc.sync, 5)
        nc.scalar.dma_start(out=wp_sb[:, :], in_=wpT[:, :])
        nc.sync.dma_start(out=sel2_sb[:, :], in_=sel2[:, :])

        nc.gpsimd.memset(den_sb[:, :], 1.0)
        for kt in range(NT):
            vv = ves_sb[:, kt * VS:(kt + 1) * VS].rearrange("p (h e) -> p h e", e=65)
            nc.gpsimd.memset(vv[:, 0:12, 64:65], 1.0)

        # ---- helpers
        QK_ORD = [0, 6, 1, 7, 2, 8, 3, 9, 4, 10, 5, 11]

        def qk_mtile_half(mt, qb, warm_between=0):
            pos = QK_ORD.index(mt)
            mp, i = pos // 2, pos % 2
            ps = psmall.tile([128, 512], F32, tag="mm", name=f"qk{mt}_{qb}")
            for ct in range(CT):
                nc.tensor.matmul(
                    ps[:, :],
                    lhsT=wqk_sb[:, mp * 1536 + ct * 256 + i * 128: mp * 1536 + ct * 256 + i * 128 + 128],
                    rhs=xT_sb[:, ct * N + qb * 512: ct * N + qb * 512 + 512],
                    start=(ct == 0), stop=(ct == CT - 1),
                    skip_group_check=(warm_between > 0),
                )
                if warm_between and ct < CT - 1:
                    warm_fill(warm_between)
            nc.vector.tensor_copy(qkT_sb[:, mt * N + qb * 512: mt * N + qb * 512 + 512], ps[:, :])

        def v_ntile(nt):
            """Compute V natural rows [nt*128, +128] and scatter into ves (+ones cols)."""
            for vb in range(2):
                ps = psmall.tile([128, 512], F32, tag="mm", name=f"v{nt}_{vb}")
                for ct in range(CT):
                    nc.tensor.matmul(
                        ps[:, 0:384],
                        lhsT=xT_sb[:, ct * N + nt * 128: ct * N + (nt + 1) * 128],
                        rhs=wv_sb[:, ct * C + vb * 384: ct * C + (vb + 1) * 384],
                        start=(ct == 0), stop=(ct == CT - 1),
                    )
                dst = ves_sb[:, nt * VS:(nt + 1) * VS].rearrange("p (h e) -> p h e", e=65)
                nc.vector.tensor_copy(
                    dst[:, vb * 6:(vb + 1) * 6, 0:64],
                    ps[:, 0:384].rearrange("p (h e) -> p h e", e=64),
                )

        def q_slice(h, qb):
            po = (h % 2) * 64
            return qkT_sb[po:po + 64, (h // 2) * N + qb * 512: (h // 2) * N + qb * 512 + 512]

        def q_full(h):
            po = (h % 2) * 64
            return qkT_sb[po:po + 64, (h // 2) * N: (h // 2) * N + N]

        def k_slice(h, kt):
            po = (h % 2) * 64
            base = (6 + h // 2) * N + kt * 128
            return qkT_sb[po:po + 64, base: base + 128]

        # pipeline state
        y_tiles = {}
        pt_kt = {}         # (pair, kt, j) -> [128, 1024] bf16 exp tile
        att_tiles = {}     # pair -> [128, 2048] bf16 (rows 0-63 numerators, row 64 denominators)

        def scores_and_exp(p, kt):
            h0, h1 = 2 * p, 2 * p + 1
            ps0 = pbig.tile([128, 1024], F32, tag="big")
            ps1 = pbig.tile([128, 1024], F32, tag="big")
            for qb in range(QB):
                nc.tensor.matmul(ps0[:, qb * 512: qb * 512 + 512], lhsT=k_slice(h0, kt),
                                 rhs=q_slice(h0, qb), start=True, stop=True)
                nc.tensor.matmul(ps1[:, qb * 512: qb * 512 + 512], lhsT=k_slice(h1, kt),
                                 rhs=q_slice(h1, qb), start=True, stop=True)
            pt0 = ptp.tile([128, 1024], BF, tag="pt", name=f"pt{p}_{kt}a")
            pt1 = ptp.tile([128, 1024], BF, tag="pt", name=f"pt{p}_{kt}b")
            pt_kt[(p, kt, 0)], pt_kt[(p, kt, 1)] = pt0, pt1
            nc.scalar.activation(pt0[:, :], ps0[:, :], EXP)
            nc.scalar.activation(pt1[:, :], ps1[:, :], EXP)

        po_open = {}

        def attn_burst_half(p, j, qb, second, evac=None):
            """Half of an attn@V accumulation group (4 MMs). Group closes and
            evacuates on the second half; den row DMAs per (j, qb) half."""
            h = 2 * p + j
            if not second:
                po_open[(p, j, qb)] = psmall.tile([65, 512], F32, tag="mm", name=f"po{h}_{qb}")
            po = po_open[(p, j, qb)]
            k0 = 4 if second else 0
            for kt in range(k0, k0 + 4):
                nc.tensor.matmul(
                    po[:, :],
                    lhsT=ves_sb[:, kt * VS + h * 65: kt * VS + h * 65 + 65],
                    rhs=pt_kt[(p, kt, j)][:, qb * 512: qb * 512 + 512],
                    start=(kt == 0), stop=(kt == NT - 1),
                )
            if second:
                po_open.pop((p, j, qb))
                if p not in att_tiles:
                    att_tiles[p] = attp.tile([128, 2 * N], BF, tag="att", name=f"att{p}")
                att_t = att_tiles[p]
                if evac == "act":
                    nc.scalar.copy(
                        att_t[0:65, j * 1024 + qb * 512: j * 1024 + qb * 512 + 512], po[:, :])
                else:
                    nc.vector.tensor_copy(
                        att_t[0:65, j * 1024 + qb * 512: j * 1024 + qb * 512 + 512], po[:, :])
                nc.sync.dma_start(
                    out=den_sb[h:h + 1, qb * 512: qb * 512 + 512],
                    in_=att_t[64:65, j * 1024 + qb * 512: j * 1024 + qb * 512 + 512])
                if qb == 1:
                    for kt in range(NT):
                        pt_kt.pop((p, kt, j))

        def attn_burst(p, j, qb, evac=None):
            attn_burst_half(p, j, qb, False)
            attn_burst_half(p, j, qb, True, evac=evac)

        def recip_half(qb):
            # full-tile ops: partition bases other than 0/32/64/96 are illegal,
            # so recompute all 12 rows (unwritten rows hold memset 1.0)
            s = slice(qb * 512, qb * 512 + 512)
            nc.vector.tensor_copy(denf_sb[:, s], den_sb[:, s])
            nc.vector.reciprocal_approx_fast(recip_sb[:, s], denf_sb[:, s])
            nc.vector.tensor_copy(rb_sb[:, s], recip_sb[:, s])

        def recip_pair(p):
            recip_half(0)
            recip_half(1)

        def divide_qb(p, qb):
            """One broadcast matmul covers both heads of the pair: bc rows 0-63
            hold recip[2p], rows 64-127 hold recip[2p+1]."""
            att_t = att_tiles[p]
            bc = psmall.tile([128, 512], F32, tag="mm", name=f"bc{p}_{qb}")
            nc.tensor.matmul(
                bc[:, :],
                lhsT=sel2_sb[0:12, p * 128:(p + 1) * 128],
                rhs=rb_sb[0:12, qb * 512: qb * 512 + 512],
                start=True, stop=True)
            for j in range(2):
                po = j * 64
                nc.vector.tensor_mul(
                    attf_sb[po:po + 64, p * N + qb * 512: p * N + qb * 512 + 512],
                    att_t[0:64, j * 1024 + qb * 512: j * 1024 + qb * 512 + 512],
                    bc[po:po + 64, :])
            if qb == 1:
                att_tiles.pop(p)

        def proj_a(nt, mb, evac=None, pool=None):
            """ct0-3 partial of the output projection -> yA."""
            if pool is None:
                pool = psmall
            tg = "mm" if pool is psmall else "big"
            ps = pool.tile([128, 512], F32, tag=tg, name=f"ya{nt}_{mb}")
            for ct in range(4):
                nc.tensor.matmul(
                    ps[:, 0:384],
                    lhsT=attf_sb[:, ct * N + nt * 128: ct * N + (nt + 1) * 128],
                    rhs=wp_sb[:, ct * C + mb * 384: ct * C + (mb + 1) * 384],
                    start=(ct == 0), stop=(ct == 3),
                )
            if evac == "act":
                nc.scalar.copy(yA_sb[:, nt * C + mb * 384: nt * C + (mb + 1) * 384], ps[:, 0:384])
            else:
                nc.vector.tensor_copy(yA_sb[:, nt * C + mb * 384: nt * C + (mb + 1) * 384], ps[:, 0:384])

        def proj_b(nt, mb, k):
            """ct4-5 closer + add partial + bf16 output DMA."""
            ps = pbig.tile([128, 512], F32, tag="big", name=f"yb{nt}_{mb}")
            for ci, ct in enumerate((4, 5)):
                nc.tensor.matmul(
                    ps[:, 0:384],
                    lhsT=attf_sb[:, ct * N + nt * 128: ct * N + (nt + 1) * 128],
                    rhs=wp_sb[:, ct * C + mb * 384: ct * C + (mb + 1) * 384],
                    start=(ci == 0), stop=(ci == 1),
                )
            if mb == 0:
                y_tiles[nt] = yp.tile([128, 768], BF, tag="y", name=f"y{nt}")
            y_t = y_tiles[nt]
            if k % 2 == 0 or nt >= 6:
                nc.vector.tensor_add(y_t[:, mb * 384:(mb + 1) * 384],
                                     yA_sb[:, nt * C + mb * 384: nt * C + (mb + 1) * 384], ps[:, 0:384])
            else:
                tmp = tmpp.tile([128, 384], F32, tag="tmp", name=f"t{nt}_{mb}")
                nc.scalar.copy(tmp[:, :], ps[:, 0:384])
                nc.gpsimd.tensor_add(y_t[:, mb * 384:(mb + 1) * 384],
                                     yA_sb[:, nt * C + mb * 384: nt * C + (mb + 1) * 384], tmp[:, :])
            if mb == 1:
                deng = nc.sync if (nt % 2 == 0 or nt == 7) else nc.scalar
                deng.dma_start(out=out[nt * 128:(nt + 1) * 128, :], in_=y_t[:, :])

        # ---- emission schedule
        # head: qk tiles for pair 0 with warm-fill between DMA-paced matmuls
        qk_mtile_half(0, 0, warm_between=2)
        qk_mtile_half(6, 0, warm_between=1)
        qk_mtile_half(0, 1)
        qk_mtile_half(6, 1)
        warm_fill(10)

        # pair 0: scores early (V weights still in flight), V + qk tiles late
        for kt in range(NT):
            scores_and_exp(0, kt)
            if kt >= 4:
                v_ntile(2 * (kt - 4))
                v_ntile(2 * (kt - 4) + 1)
            if kt == 6:
                qk_mtile_half(1, 0)
                qk_mtile_half(1, 1)
            elif kt == 7:
                qk_mtile_half(7, 0)
                qk_mtile_half(7, 1)

        # pairs 1..5: attn@V of p-1 first in each slot, then scores of p, then
        # qk tiles for p+1 in late slots (pairs 1-4) or proj partials (pair 5)
        for p in range(1, PAIRS):
            last = (p == PAIRS - 1)
            for kt in range(NT):
                if kt < 4:
                    attn_burst(p - 1, kt // 2, kt % 2)
                elif kt == 4:
                    recip_pair(p - 1)
                elif kt in (5, 6):
                    divide_qb(p - 1, kt - 5)
                scores_and_exp(p, kt)
                if not last:
                    if kt >= 4:
                        mt, qb = [(p + 1, 0), (p + 1, 1), (p + 7, 0), (p + 7, 1)][kt - 4]
                        qk_mtile_half(mt, qb)
                else:
                    if kt >= 2:
                        proj_a(kt - 2, 0)
                        proj_a(kt - 2, 1)

        # ---- drain: qb0 attn groups first so the divide chain and the proj
        # closers pipeline per qb half
        p = PAIRS - 1
        attn_burst(p, 0, 0)
        attn_burst(p, 1, 0)
        proj_a(6, 0)
        proj_a(6, 1)
        attn_burst(p, 0, 1)
        attn_burst(p, 1, 1)
        proj_a(7, 0)
        proj_a(7, 1)
        recip_half(0)
        divide_qb(p, 0)
        k = 0
        for nt in range(4):
            for mb in range(2):
                proj_b(nt, mb, k)
                k += 1
        recip_half(1)
        divide_qb(p, 1)
        for nt in range(4, NT):
            for mb in range(2):
                proj_b(nt, mb, k)
                k += 1

    nc.compile()
    return nc


_CACHE = {}


def _prep_inputs(x, w_qkv, w_proj):
    bf = ml_dtypes.bfloat16
    w = np.array(w_qkv, dtype=np.float32, copy=True)
    w[:C] *= SCALE
    wqkT = w[:2 * C].T.astype(bf)                                # [C, 2C]
    ord_ = [0, 6, 1, 7, 2, 8, 3, 9, 4, 10, 5, 11]
    # p-major m-pair blocks: wqk[mp, p, ct*256 + i*128 + c] with m-tiles
    # (ord_[2mp], ord_[2mp+1])
    wqk = np.zeros((PAIRS, 128, 1536), dtype=bf)
    for b in range(PAIRS):
        for ct in range(CT):
            for i in range(2):
                mt = ord_[2 * b + i]
                wqk[b, :, ct * 256 + i * 128: ct * 256 + (i + 1) * 128] = \
                    wqkT[ct * 128:(ct + 1) * 128, mt * 128:(mt + 1) * 128]
    # p-major [128, ct*C] images of wv/wp: row p, block ct = wT[ct*128+p, :]
    wvT_n = w[2 * C:].T.astype(bf)                               # [C, C]
    wpT_n = np.asarray(w_proj).T.astype(np.float32).astype(bf)   # [C, C]
    wvT_t = np.ascontiguousarray(wvT_n.reshape(CT, 128, C).transpose(1, 0, 2).reshape(128, CT * C))
    wpT_t = np.ascontiguousarray(wpT_n.reshape(CT, 128, C).transpose(1, 0, 2).reshape(128, CT * C))
    sel2 = np.zeros((12, PAIRS * 128), dtype=bf)
    for p in range(PAIRS):
        sel2[2 * p, p * 128:p * 128 + 64] = 1.0
        sel2[2 * p + 1, p * 128 + 64:(p + 1) * 128] = 1.0
    maps = []
    for b in range(B):
        xT = np.asarray(x[b]).T.astype(bf)                       # [C, N]
        xTt = np.ascontiguousarray(xT.reshape(CT, 128, N).transpose(1, 0, 2).reshape(128, CT * N))
        maps.append({
            "xTt": xTt, "wqk": wqk, "wvT": wvT_t, "wpT": wpT_t, "sel2": sel2,
        })
    return maps


def kernel(x, w_qkv, w_proj, b_proj):
    from concourse.bass_utils import run_bass_kernel_spmd

    if "nc" not in _CACHE:
        _CACHE["nc"] = build_nc()
    nc = _CACHE["nc"]
    in_maps = _prep_inputs(x, w_qkv, w_proj)
    res = run_bass_kernel_spmd(nc, in_maps, core_ids=list(range(B)))
    y = np.stack([np.asarray(res.results[i]["out"], dtype=np.float32) for i in range(B)])
    y = y + np.asarray(b_proj, dtype=np.float32)[None, None, :]
    return y.astype(np.float32)


if __name__ == "__main__":
    nc = build_nc()
    print("build OK")
